# revision 19
# baseline (speedup 1.0000x reference)
"""Trainium2 Bass kernel for nn_MultiHeadLatentAttention_82068235092052.

Reference computation (B=2, S=4096, E=4096, H=32, D=128):
    q = hs @ wq.T + bq   -> [B,S,H,D]     (wq/bq are fp8-roundtripped fp32)
    k = hs @ wk.T + bk
    v = hs @ wv.T + bv
    (latent = hs @ wl.T + bl is computed but UNUSED -> skipped entirely)
    scores  = einsum('bshd,bstd->bsht', q, k) / sqrt(D)   # attention over HEADS per position
    probs   = softmax(scores, -1)
    context = einsum('bsht,bstd->bshd', probs, v).reshape(B,S,E)

Strategy: data-parallel over the 8192 positions across 8 cores (1024 each).
Per core the positions are processed in 5 slabs (256/256/256/192/64); the
per-position 32x32 head-attention of slab s-1 is interleaved into the
projection matmuls of slab s, so only the tiny last slab's attention is
exposed at the end.

Projections stream the fused W[12288,4096] weights as fp8-e4m3 (lossless:
the reference weights are fp8-roundtripped) into mixed fp8xbf16 matmuls.
q/k/v land pos-major [d, pos, head]; per 16-position block the scores are
computed with 4 cross-position matmuls (N=128, 4 positions each), the
off-diagonal cross terms are killed by a block-diagonal mask fused into the
tensor_tensor_reduce that also produces the softmax denominators (zsum).
probs stay UNNORMALIZED on device; zsum ships to the host, which divides in
fp32 while assembling the output.
"""

import os
import sys

import numpy as np

sys.path.insert(0, "/opt/trn_rl_repo")

import ml_dtypes

import concourse.bacc as bacc
import concourse.bass as bass
import concourse.tile as tile
from concourse import mybir
from concourse.masks import make_identity

# Problem constants (hardcoded; kernel.py must be self-contained).
B, S, E = 2, 4096, 4096
H, D = 32, 128
P_TOT = B * S            # 8192 positions
N_CORES = 8
P_CORE = P_TOT // N_CORES  # 1024 positions per core
FT = 3 * H                 # 96 feature tiles (q, k, v concatenated)
KT = E // 128              # 32 contraction tiles

SLABS = [256, 256, 256, 192, 64]
assert sum(SLABS) == P_CORE and all(s % 16 == 0 for s in SLABS)
SMAX = max(SLABS)

BF16 = mybir.dt.bfloat16
F32 = mybir.dt.float32
FP8 = mybir.dt.float8e4

_CACHED_NC = None


def build_nc():
    """Build the per-core Bass program (same program on all 8 cores)."""
    nc = bacc.Bacc(
        "TRN2",
        target_bir_lowering=False,
        debug=False,
        enable_asserts=True,
        num_devices=1,
    )

    xt = nc.dram_tensor("xt", [128, KT, P_CORE], BF16, kind="ExternalInput").ap()
    wt = nc.dram_tensor("wt", [FT, 128, KT * 128], FP8, kind="ExternalInput").ap()
    bias = nc.dram_tensor("bias", [128, FT], F32, kind="ExternalInput").ap()
    maskd = nc.dram_tensor("maskd", [128, 128], BF16, kind="ExternalInput").ap()
    ctx_out = nc.dram_tensor("ctx", [128, P_CORE, H], BF16, kind="ExternalOutput").ap()
    zsum_out = nc.dram_tensor("zsum", [128, P_CORE // 4], F32, kind="ExternalOutput").ap()

    from contextlib import ExitStack

    with tile.TileContext(nc) as tc, ExitStack() as stack:
        const = stack.enter_context(tc.tile_pool(name="const", bufs=1))
        xtp = stack.enter_context(tc.tile_pool(name="xtp", bufs=2))
        qkvp = stack.enter_context(tc.tile_pool(name="qkvp", bufs=2))
        wp = stack.enter_context(tc.tile_pool(name="wp", bufs=3))
        zsp = stack.enter_context(tc.tile_pool(name="zsp", bufs=2))
        asb = stack.enter_context(tc.tile_pool(name="asb", bufs=3))
        ctp = stack.enter_context(tc.tile_pool(name="ctp", bufs=3))
        psp = stack.enter_context(tc.tile_pool(name="psp", bufs=2, space="PSUM"))
        scp = stack.enter_context(tc.tile_pool(name="scp", bufs=3, space="PSUM"))
        cdp = stack.enter_context(tc.tile_pool(name="cdp", bufs=2, space="PSUM"))

        bias_sb = const.tile([128, FT], F32)
        nc.sync.dma_start(bias_sb, bias)
        mask_sb = const.tile([128, 128], BF16)
        nc.sync.dma_start(mask_sb, maskd)

        inv_sqrt_d = 1.0 / float(np.sqrt(D))

        def emit_attn_front(slab_tiles, blk):
            """QK + VT + softmax front half of one 16-position block."""
            q_sb, k_sb, v_sb, zs_sb, sstart = slab_tiles
            p0 = blk * 16
            scores = scp.tile([128, 4, 128], F32, tag="sc")
            for g in range(4):
                nc.tensor.matmul(
                    scores[:, g, :],
                    lhsT=q_sb[:, p0 + 4 * g:p0 + 4 * g + 4, :],
                    rhs=k_sb[:, p0 + 4 * g:p0 + 4 * g + 4, :],
                    start=True,
                    stop=True,
                )
            vt_sb = asb.tile([128, 4, 128], BF16, tag="vts")
            for g in range(4):
                nc.sync.dma_start_transpose(
                    vt_sb[:, g, :],
                    v_sb[:, p0 + 4 * g:p0 + 4 * g + 4, :].opt(),
                )
            exp_sb = asb.tile([128, 4, 128], BF16, tag="exp")
            nc.scalar.activation(
                exp_sb, scores, mybir.ActivationFunctionType.Exp, scale=inv_sqrt_d
            )
            masked = asb.tile([128, 4, 128], BF16, tag="mk")
            nc.vector.tensor_tensor(
                masked,
                exp_sb,
                mask_sb[:, None, :].to_broadcast((128, 4, 128)),
                mybir.AluOpType.mult,
            )
            nc.vector.tensor_reduce(
                zs_sb[:, 4 * blk:4 * blk + 4],
                masked,
                axis=mybir.AxisListType.X,
                op=mybir.AluOpType.add,
            )
            probsT = asb.tile([128, 4, 128], BF16, tag="pt")
            nc.vector.transpose(probsT, masked)  # block-diagonal -> true transpose
            return (probsT, vt_sb, sstart, p0)

        def emit_attn_back(pend):
            """PV + ctx output of a previously fronted block."""
            probsT, vt_sb, sstart, p0 = pend
            ctd = cdp.tile([128, 4, 128], F32, tag="ctd")
            for g in range(4):
                nc.tensor.matmul(
                    ctd[:, g, :],
                    lhsT=vt_sb[:, g, :],
                    rhs=probsT[:, g, :],
                    start=True,
                    stop=True,
                )
            ct_blk = ctp.tile([128, 16, H], BF16, tag="ct")
            nc.scalar.copy(ct_blk, ctd)
            nc.sync.dma_start(
                ctx_out[:, sstart + p0:sstart + p0 + 16, :], ct_blk
            )

        def fetch_xt(si):
            s0 = sum(SLABS[:si])
            xt_sb = xtp.tile([128, KT, SLABS[si]], BF16, tag="xt")
            for kc in range(4):
                nc.sync.dma_start(
                    xt_sb[:, 8 * kc:8 * kc + 8, :],
                    xt[:, 8 * kc:8 * kc + 8, s0:s0 + SLABS[si]],
                )
            return xt_sb

        prev_tiles = None
        pending = None
        sstart = 0
        next_xt = fetch_xt(0)
        for si, SL in enumerate(SLABS):
            xt_sb = next_xt
            q_sb = qkvp.tile([128, SL, H], BF16, tag="q")
            k_sb = qkvp.tile([128, SL, H], BF16, tag="k")
            v_sb = qkvp.tile([128, SL, H], BF16, tag="v")
            zs_sb = zsp.tile([128, SL // 4], F32, tag="zs")
            dsts = (q_sb, k_sb, v_sb)

            nblk_prev = SLABS[si - 1] // 16 if si > 0 else 0
            attn_j = 0
            for ft in range(FT):
                w_sb = wp.tile([128, KT, 128], FP8, tag="w")
                nc.sync.dma_start(w_sb, wt[ft].rearrange("p (a b) -> p a b", a=KT))
                ps = psp.tile([128, SL], F32, tag="ps", padded_shape=[128, 512])
                for kt in range(KT):
                    nc.tensor.matmul(
                        ps,
                        lhsT=w_sb[:, kt, :],
                        rhs=xt_sb[:, kt, :],
                        start=(kt == 0),
                        stop=(kt == KT - 1),
                    )
                # bias add (per-partition scalar) + cast to bf16, PSUM -> SBUF
                nc.vector.tensor_scalar(
                    out=dsts[ft // H][:, :, ft % H],
                    in0=ps,
                    scalar1=bias_sb[:, ft:ft + 1],
                    scalar2=None,
                    op0=mybir.AluOpType.add,
                )
                if ft == 8 and si + 1 < len(SLABS):
                    next_xt = fetch_xt(si + 1)
                # interleave previous slab's attention blocks across the ft loop
                while attn_j < nblk_prev * (ft + 1) // FT:
                    front = emit_attn_front(prev_tiles, attn_j)
                    if pending is not None:
                        emit_attn_back(pending)
                    pending = front
                    attn_j += 1
            if si > 0:
                nc.sync.dma_start(
                    zsum_out[:, prev_tiles[4] // 4:(prev_tiles[4] + SLABS[si - 1]) // 4],
                    prev_tiles[3],
                )
            prev_tiles = (q_sb, k_sb, v_sb, zs_sb, sstart)
            sstart += SL

        # last slab's attention (the only non-overlapped part)
        for blk in range(SLABS[-1] // 16):
            front = emit_attn_front(prev_tiles, blk)
            if pending is not None:
                emit_attn_back(pending)
            pending = front
        emit_attn_back(pending)
        nc.sync.dma_start(
            zsum_out[:, prev_tiles[4] // 4:(prev_tiles[4] + SLABS[-1]) // 4],
            prev_tiles[3],
        )

    nc.compile()
    return nc


def get_nc():
    global _CACHED_NC
    if _CACHED_NC is None:
        _CACHED_NC = build_nc()
    return _CACHED_NC


def prep_inputs(hidden_states, wq, bq, wk, bk, wv, bv):
    """Host-side layout prep. Returns per-core input maps."""
    bf16 = ml_dtypes.bfloat16

    # X^T tiled: [ipart, kt, p] with p the global position index
    xt_all = (
        np.ascontiguousarray(hidden_states.reshape(P_TOT, E).T)
        .astype(bf16)
        .reshape(KT, 128, P_TOT)
        .transpose(1, 0, 2)
    )  # [128, KT, 8192] (view)

    # Fused weight W[12288, 4096] -> W^T tiled [ft, ipart, kt*128 + f].
    # Weights are fp8-e4m3 roundtripped, so fp8 storage is lossless.
    wcat = np.concatenate([wq, wk, wv], axis=0)  # [3E, E]
    wt = (
        np.ascontiguousarray(wcat.T)
        .astype(ml_dtypes.float8_e4m3)
        .reshape(KT, 128, FT, 128)
        .transpose(2, 1, 0, 3)
    )
    wt = np.ascontiguousarray(wt).reshape(FT, 128, KT * 128)

    bias_cols = np.ascontiguousarray(
        np.concatenate([bq, bk, bv]).astype(np.float32).reshape(FT, 128).T
    )  # [128, FT]

    mask = np.zeros((128, 128), dtype=bf16)
    for p in range(4):
        mask[32 * p:32 * p + 32, 32 * p:32 * p + 32] = 1.0

    in_maps = []
    for c in range(N_CORES):
        xt_c = np.ascontiguousarray(xt_all[:, :, c * P_CORE:(c + 1) * P_CORE])
        in_maps.append({"xt": xt_c, "wt": wt, "bias": bias_cols, "maskd": mask})
    return in_maps


def normalize_shard(ctx_u, zsum):
    """ctx_u [128, P_CORE, H] bf16 (d, pos, h) unnormalized; zsum [128, P_CORE//4].

    Returns normalized [P_CORE, E] fp32. z for position pos, head h lives at
    zsum[32*(pos%4)+h, pos//4]."""
    ctx = np.asarray(ctx_u).astype(np.float32).transpose(1, 2, 0)  # [pos, h, d]
    z = np.asarray(zsum).astype(np.float32)  # [128, P_CORE//4]
    z = z.reshape(4, 32, P_CORE // 4).transpose(2, 0, 1).reshape(P_CORE, 32)
    return (ctx / z[:, :, None]).reshape(P_CORE, E)


def assemble_output(ctxs, zsums):
    out = np.empty((P_TOT, E), dtype=np.float32)
    for c in range(N_CORES):
        out[c * P_CORE:(c + 1) * P_CORE] = normalize_shard(ctxs[c], zsums[c])
    return out.reshape(B, S, E)


def kernel(**inputs):
    from concourse.bass_utils import run_bass_kernel_spmd

    nc = get_nc()
    in_maps = prep_inputs(
        inputs["hidden_states"],
        inputs["wq"], inputs["bq"],
        inputs["wk"], inputs["bk"],
        inputs["wv"], inputs["bv"],
    )
    res = run_bass_kernel_spmd(nc, in_maps, core_ids=list(range(N_CORES)))
    ctxs = [np.asarray(r["ctx"]).reshape(128, P_CORE, H) for r in res.results]
    zsums = [np.asarray(r["zsum"]).reshape(128, P_CORE // 4) for r in res.results]
    return assemble_output(ctxs, zsums)


# revision 23
# speedup vs baseline: 1.1221x; 1.1221x over previous
"""Trainium2 Bass kernel for nn_MultiHeadLatentAttention_82068235092052.

Reference computation (B=2, S=4096, E=4096, H=32, D=128):
    q = hs @ wq.T + bq   -> [B,S,H,D]     (wq/bq are fp8-roundtripped fp32)
    k = hs @ wk.T + bk
    v = hs @ wv.T + bv
    (latent = hs @ wl.T + bl is computed but UNUSED -> skipped entirely)
    scores  = einsum('bshd,bstd->bsht', q, k) / sqrt(D)   # attention over HEADS per position
    probs   = softmax(scores, -1)
    context = einsum('bsht,bstd->bshd', probs, v).reshape(B,S,E)

Strategy: data-parallel over the 8192 positions across 8 cores (1024 each).
Per core the positions are processed in 5 slabs (256/256/256/192/64); the
per-position 32x32 head-attention of slab s-1 is interleaved into the
projection matmuls of slab s, so only the tiny last slab's attention is
exposed at the end.

Projections stream the fused W[12288,4096] weights as fp8-e4m3 (lossless:
the reference weights are fp8-roundtripped) into mixed fp8xbf16 matmuls.
q/k/v land pos-major [d, pos, head]; per 16-position block the scores are
computed with 4 cross-position matmuls (N=128, 4 positions each), the
off-diagonal cross terms are killed by a block-diagonal mask fused into the
tensor_tensor_reduce that also produces the softmax denominators (zsum).
probs stay UNNORMALIZED on device; zsum ships to the host, which divides in
fp32 while assembling the output.
"""

import os
import sys

import numpy as np

sys.path.insert(0, "/opt/trn_rl_repo")

import ml_dtypes

import concourse.bacc as bacc
import concourse.bass as bass
import concourse.tile as tile
from concourse import mybir
from concourse.masks import make_identity

# Problem constants (hardcoded; kernel.py must be self-contained).
B, S, E = 2, 4096, 4096
H, D = 32, 128
P_TOT = B * S            # 8192 positions
N_CORES = 8
P_CORE = P_TOT // N_CORES  # 1024 positions per core
FT = 3 * H                 # 96 feature tiles (q, k, v concatenated)
KT = E // 128              # 32 contraction tiles

SLABS = [256, 256, 256, 192, 64]
assert sum(SLABS) == P_CORE and all(s % 16 == 0 for s in SLABS)
SMAX = max(SLABS)

BF16 = mybir.dt.bfloat16
F32 = mybir.dt.float32
FP8 = mybir.dt.float8e4

_CACHED_NC = None


def build_nc():
    """Build the per-core Bass program (same program on all 8 cores)."""
    nc = bacc.Bacc(
        "TRN2",
        target_bir_lowering=False,
        debug=False,
        enable_asserts=True,
        num_devices=1,
    )

    xt = nc.dram_tensor("xt", [128, KT, P_CORE], BF16, kind="ExternalInput").ap()
    wt = nc.dram_tensor("wt", [FT, 128, KT * 128], FP8, kind="ExternalInput").ap()
    bias = nc.dram_tensor("bias", [128, FT], F32, kind="ExternalInput").ap()
    maskd = nc.dram_tensor("maskd", [128, 128], BF16, kind="ExternalInput").ap()
    ctx_out = nc.dram_tensor("ctx", [128, P_CORE, H], BF16, kind="ExternalOutput").ap()
    zsum_out = nc.dram_tensor("zsum", [128, P_CORE // 4], F32, kind="ExternalOutput").ap()

    from contextlib import ExitStack

    with tile.TileContext(nc) as tc, ExitStack() as stack:
        const = stack.enter_context(tc.tile_pool(name="const", bufs=1))
        xtp = stack.enter_context(tc.tile_pool(name="xtp", bufs=2))
        qkvp = stack.enter_context(tc.tile_pool(name="qkvp", bufs=2))
        wp = stack.enter_context(tc.tile_pool(name="wp", bufs=3))
        zsp = stack.enter_context(tc.tile_pool(name="zsp", bufs=2))
        asb = stack.enter_context(tc.tile_pool(name="asb", bufs=3))
        ctp = stack.enter_context(tc.tile_pool(name="ctp", bufs=3))
        psp = stack.enter_context(tc.tile_pool(name="psp", bufs=2, space="PSUM"))
        scp = stack.enter_context(tc.tile_pool(name="scp", bufs=2, space="PSUM"))
        vtp = stack.enter_context(tc.tile_pool(name="vtp", bufs=2, space="PSUM"))
        cdp = stack.enter_context(tc.tile_pool(name="cdp", bufs=2, space="PSUM"))

        identity = const.tile([128, 128], BF16)
        make_identity(nc, identity)
        bias_sb = const.tile([128, FT], F32)
        nc.sync.dma_start(bias_sb, bias)
        mask_sb = const.tile([128, 128], BF16)
        nc.sync.dma_start(mask_sb, maskd)

        inv_sqrt_d = 1.0 / float(np.sqrt(D))

        def emit_attn_front(slab_tiles, blk):
            """QK + VT + softmax front half of one 16-position block."""
            q_sb, k_sb, v_sb, zs_sb, sstart = slab_tiles
            p0 = blk * 16
            scores = scp.tile([128, 4, 128], F32, tag="sc")
            for g in range(4):
                nc.tensor.matmul(
                    scores[:, g, :],
                    lhsT=q_sb[:, p0 + 4 * g:p0 + 4 * g + 4, :],
                    rhs=k_sb[:, p0 + 4 * g:p0 + 4 * g + 4, :],
                    start=True,
                    stop=True,
                )
            vt_ps = vtp.tile([128, 4, 128], BF16, tag="vt", padded_shape=[128, 4, 256])
            for g in range(4):
                nc.tensor.transpose(
                    vt_ps[:, g, :],
                    v_sb[:, p0 + 4 * g:p0 + 4 * g + 4, :].opt(),
                    identity,
                )
            exp_sb = asb.tile([128, 4, 128], BF16, tag="exp")
            nc.scalar.activation(
                exp_sb, scores, mybir.ActivationFunctionType.Exp, scale=inv_sqrt_d
            )
            masked = asb.tile([128, 4, 128], BF16, tag="mk")
            nc.vector.tensor_tensor(
                masked,
                exp_sb,
                mask_sb[:, None, :].to_broadcast((128, 4, 128)),
                mybir.AluOpType.mult,
            )
            nc.vector.tensor_reduce(
                zs_sb[:, 4 * blk:4 * blk + 4],
                masked,
                axis=mybir.AxisListType.X,
                op=mybir.AluOpType.add,
            )
            probsT = asb.tile([128, 4, 128], BF16, tag="pt")
            nc.vector.transpose(probsT, masked)  # block-diagonal -> true transpose
            vt_sb = asb.tile([128, 4, 128], BF16, tag="vts")
            nc.scalar.copy(vt_sb, vt_ps)
            return (probsT, vt_sb, sstart, p0)

        def emit_attn_back(pend):
            """PV + ctx output of a previously fronted block."""
            probsT, vt_sb, sstart, p0 = pend
            ctd = cdp.tile([128, 4, 128], F32, tag="ctd")
            for g in range(4):
                nc.tensor.matmul(
                    ctd[:, g, :],
                    lhsT=vt_sb[:, g, :],
                    rhs=probsT[:, g, :],
                    start=True,
                    stop=True,
                )
            ct_blk = ctp.tile([128, 16, H], BF16, tag="ct")
            nc.scalar.copy(ct_blk, ctd)
            nc.sync.dma_start(
                ctx_out[:, sstart + p0:sstart + p0 + 16, :], ct_blk
            )

        def fetch_xt(si):
            s0 = sum(SLABS[:si])
            xt_sb = xtp.tile([128, KT, SLABS[si]], BF16, tag="xt")
            for kc in range(4):
                nc.sync.dma_start(
                    xt_sb[:, 8 * kc:8 * kc + 8, :],
                    xt[:, 8 * kc:8 * kc + 8, s0:s0 + SLABS[si]],
                )
            return xt_sb

        prev_tiles = None
        pending = None
        sstart = 0
        next_xt = fetch_xt(0)
        for si, SL in enumerate(SLABS):
            xt_sb = next_xt
            q_sb = qkvp.tile([128, SL, H], BF16, tag="q")
            k_sb = qkvp.tile([128, SL, H], BF16, tag="k")
            v_sb = qkvp.tile([128, SL, H], BF16, tag="v")
            zs_sb = zsp.tile([128, SL // 4], F32, tag="zs")
            dsts = (q_sb, k_sb, v_sb)

            nblk_prev = SLABS[si - 1] // 16 if si > 0 else 0
            attn_j = 0
            for ft in range(FT):
                w_sb = wp.tile([128, KT, 128], FP8, tag="w")
                wsrc = wt[ft].rearrange("p (a b) -> p a b", a=KT)
                if si == 0 and ft == 0:
                    for kc in range(4):
                        nc.sync.dma_start(
                            w_sb[:, 8 * kc:8 * kc + 8, :], wsrc[:, 8 * kc:8 * kc + 8, :]
                        )
                else:
                    nc.sync.dma_start(w_sb, wsrc)
                ps = psp.tile([128, SL], F32, tag="ps", padded_shape=[128, 512])
                for kt in range(KT):
                    nc.tensor.matmul(
                        ps,
                        lhsT=w_sb[:, kt, :],
                        rhs=xt_sb[:, kt, :],
                        start=(kt == 0),
                        stop=(kt == KT - 1),
                    )
                # bias add (per-partition scalar) + cast to bf16, PSUM -> SBUF
                nc.vector.tensor_scalar(
                    out=dsts[ft // H][:, :, ft % H],
                    in0=ps,
                    scalar1=bias_sb[:, ft:ft + 1],
                    scalar2=None,
                    op0=mybir.AluOpType.add,
                )
                if ft == 8 and si + 1 < len(SLABS):
                    next_xt = fetch_xt(si + 1)
                # interleave previous slab's attention blocks across the ft loop
                while attn_j < nblk_prev * (ft + 1) // FT:
                    front = emit_attn_front(prev_tiles, attn_j)
                    if pending is not None:
                        emit_attn_back(pending)
                    pending = front
                    attn_j += 1
            if si > 0:
                nc.sync.dma_start(
                    zsum_out[:, prev_tiles[4] // 4:(prev_tiles[4] + SLABS[si - 1]) // 4],
                    prev_tiles[3],
                )
            prev_tiles = (q_sb, k_sb, v_sb, zs_sb, sstart)
            sstart += SL

        # last slab's attention (the only non-overlapped part)
        for blk in range(SLABS[-1] // 16):
            front = emit_attn_front(prev_tiles, blk)
            if pending is not None:
                emit_attn_back(pending)
            pending = front
        emit_attn_back(pending)
        nc.sync.dma_start(
            zsum_out[:, prev_tiles[4] // 4:(prev_tiles[4] + SLABS[-1]) // 4],
            prev_tiles[3],
        )

    nc.compile()
    return nc


def get_nc():
    global _CACHED_NC
    if _CACHED_NC is None:
        _CACHED_NC = build_nc()
    return _CACHED_NC


def prep_inputs(hidden_states, wq, bq, wk, bk, wv, bv):
    """Host-side layout prep. Returns per-core input maps."""
    bf16 = ml_dtypes.bfloat16

    # X^T tiled: [ipart, kt, p] with p the global position index
    xt_all = (
        np.ascontiguousarray(hidden_states.reshape(P_TOT, E).T)
        .astype(bf16)
        .reshape(KT, 128, P_TOT)
        .transpose(1, 0, 2)
    )  # [128, KT, 8192] (view)

    # Fused weight W[12288, 4096] -> W^T tiled [ft, ipart, kt*128 + f].
    # Weights are fp8-e4m3 roundtripped, so fp8 storage is lossless.
    wcat = np.concatenate([wq, wk, wv], axis=0)  # [3E, E]
    wt = (
        np.ascontiguousarray(wcat.T)
        .astype(ml_dtypes.float8_e4m3)
        .reshape(KT, 128, FT, 128)
        .transpose(2, 1, 0, 3)
    )
    wt = np.ascontiguousarray(wt).reshape(FT, 128, KT * 128)

    bias_cols = np.ascontiguousarray(
        np.concatenate([bq, bk, bv]).astype(np.float32).reshape(FT, 128).T
    )  # [128, FT]

    mask = np.zeros((128, 128), dtype=bf16)
    for p in range(4):
        mask[32 * p:32 * p + 32, 32 * p:32 * p + 32] = 1.0

    in_maps = []
    for c in range(N_CORES):
        xt_c = np.ascontiguousarray(xt_all[:, :, c * P_CORE:(c + 1) * P_CORE])
        in_maps.append({"xt": xt_c, "wt": wt, "bias": bias_cols, "maskd": mask})
    return in_maps


def normalize_shard(ctx_u, zsum):
    """ctx_u [128, P_CORE, H] bf16 (d, pos, h) unnormalized; zsum [128, P_CORE//4].

    Returns normalized [P_CORE, E] fp32. z for position pos, head h lives at
    zsum[32*(pos%4)+h, pos//4]."""
    ctx = np.asarray(ctx_u).astype(np.float32).transpose(1, 2, 0)  # [pos, h, d]
    z = np.asarray(zsum).astype(np.float32)  # [128, P_CORE//4]
    z = z.reshape(4, 32, P_CORE // 4).transpose(2, 0, 1).reshape(P_CORE, 32)
    return (ctx / z[:, :, None]).reshape(P_CORE, E)


def assemble_output(ctxs, zsums):
    out = np.empty((P_TOT, E), dtype=np.float32)
    for c in range(N_CORES):
        out[c * P_CORE:(c + 1) * P_CORE] = normalize_shard(ctxs[c], zsums[c])
    return out.reshape(B, S, E)


def kernel(**inputs):
    from concourse.bass_utils import run_bass_kernel_spmd

    nc = get_nc()
    in_maps = prep_inputs(
        inputs["hidden_states"],
        inputs["wq"], inputs["bq"],
        inputs["wk"], inputs["bk"],
        inputs["wv"], inputs["bv"],
    )
    res = run_bass_kernel_spmd(nc, in_maps, core_ids=list(range(N_CORES)))
    ctxs = [np.asarray(r["ctx"]).reshape(128, P_CORE, H) for r in res.results]
    zsums = [np.asarray(r["zsum"]).reshape(128, P_CORE // 4) for r in res.results]
    return assemble_output(ctxs, zsums)


# revision 25
# speedup vs baseline: 1.3130x; 1.1702x over previous
"""Trainium2 Bass kernel for nn_MultiHeadLatentAttention_82068235092052.

Reference computation (B=2, S=4096, E=4096, H=32, D=128):
    q = hs @ wq.T + bq   -> [B,S,H,D]     (wq/bq are fp8-roundtripped fp32)
    k = hs @ wk.T + bk
    v = hs @ wv.T + bv
    (latent = hs @ wl.T + bl is computed but UNUSED -> skipped entirely)
    scores  = einsum('bshd,bstd->bsht', q, k) / sqrt(D)   # attention over HEADS per position
    probs   = softmax(scores, -1)
    context = einsum('bsht,bstd->bshd', probs, v).reshape(B,S,E)

Strategy: data-parallel over the 8192 positions across 8 cores (1024 each).
Per core the positions are processed in 5 slabs (256/256/256/192/64); the
per-position 32x32 head-attention of slab s-1 is interleaved into the
projection matmuls of slab s, so only the tiny last slab's attention is
exposed at the end.

Projections stream the fused W[12288,4096] weights as fp8-e4m3 (lossless:
the reference weights are fp8-roundtripped) into mixed fp8xbf16 matmuls.
q/k/v land pos-major [d, pos, head]; per 16-position block the scores are
computed with 4 cross-position matmuls (N=128, 4 positions each), the
off-diagonal cross terms are killed by a block-diagonal mask fused into the
tensor_tensor_reduce that also produces the softmax denominators (zsum).
probs stay UNNORMALIZED on device; zsum ships to the host, which divides in
fp32 while assembling the output.
"""

import os
import sys

import numpy as np

sys.path.insert(0, "/opt/trn_rl_repo")

import ml_dtypes

import concourse.bacc as bacc
import concourse.bass as bass
import concourse.tile as tile
from concourse import mybir
from concourse.masks import make_identity

# Problem constants (hardcoded; kernel.py must be self-contained).
B, S, E = 2, 4096, 4096
H, D = 32, 128
P_TOT = B * S            # 8192 positions
N_CORES = 8
P_CORE = P_TOT // N_CORES  # 1024 positions per core
FT = 3 * H                 # 96 feature tiles (q, k, v concatenated)
KT = E // 128              # 32 contraction tiles

SLABS = [256, 256, 256, 192, 64]
assert sum(SLABS) == P_CORE and all(s % 16 == 0 for s in SLABS)
SMAX = max(SLABS)

BF16 = mybir.dt.bfloat16
F32 = mybir.dt.float32
FP8 = mybir.dt.float8e4

_CACHED_NC = None


def build_nc():
    """Build the per-core Bass program (same program on all 8 cores)."""
    nc = bacc.Bacc(
        "TRN2",
        target_bir_lowering=False,
        debug=False,
        enable_asserts=True,
        num_devices=1,
    )

    xt = nc.dram_tensor("xt", [128, KT, P_CORE], BF16, kind="ExternalInput").ap()
    wt = nc.dram_tensor("wt", [FT, 128, KT * 128], FP8, kind="ExternalInput").ap()
    bias = nc.dram_tensor("bias", [128, FT], F32, kind="ExternalInput").ap()
    maskd = nc.dram_tensor("maskd", [128, 128], BF16, kind="ExternalInput").ap()
    ctx_out = nc.dram_tensor("ctx", [128, P_CORE, H], BF16, kind="ExternalOutput").ap()
    zsum_out = nc.dram_tensor("zsum", [128, P_CORE // 4], F32, kind="ExternalOutput").ap()

    from contextlib import ExitStack

    with tile.TileContext(nc) as tc, ExitStack() as stack:
        const = stack.enter_context(tc.tile_pool(name="const", bufs=1))
        xtp = stack.enter_context(tc.tile_pool(name="xtp", bufs=2))
        qkvp = stack.enter_context(tc.tile_pool(name="qkvp", bufs=2))
        wp = stack.enter_context(tc.tile_pool(name="wp", bufs=3))
        zsp = stack.enter_context(tc.tile_pool(name="zsp", bufs=2))
        asb = stack.enter_context(tc.tile_pool(name="asb", bufs=2))
        ctp = stack.enter_context(tc.tile_pool(name="ctp", bufs=3))
        psp = stack.enter_context(tc.tile_pool(name="psp", bufs=2, space="PSUM"))
        scp = stack.enter_context(tc.tile_pool(name="scp", bufs=2, space="PSUM"))
        vtp = stack.enter_context(tc.tile_pool(name="vtp", bufs=2, space="PSUM"))
        cdp = stack.enter_context(tc.tile_pool(name="cdp", bufs=2, space="PSUM"))

        identity = const.tile([128, 128], BF16)
        make_identity(nc, identity)
        bias_sb = const.tile([128, FT], F32)
        nc.sync.dma_start(bias_sb, bias)
        mask_sb = const.tile([128, 128], BF16)
        nc.sync.dma_start(mask_sb, maskd)

        inv_sqrt_d = 1.0 / float(np.sqrt(D))

        def emit_attn_front(slab_tiles, blk):
            """QK + VT + softmax front half of one 16-position block."""
            q_sb, k_sb, v_sb, zs_sb, sstart = slab_tiles
            p0 = blk * 16
            scores = scp.tile([128, 4, 128], F32, tag="sc")
            for g in range(4):
                nc.tensor.matmul(
                    scores[:, g, :],
                    lhsT=q_sb[:, p0 + 4 * g:p0 + 4 * g + 4, :],
                    rhs=k_sb[:, p0 + 4 * g:p0 + 4 * g + 4, :],
                    start=True,
                    stop=True,
                )
            vt_ps = vtp.tile([128, 4, 128], BF16, tag="vt", padded_shape=[128, 4, 256])
            for g in range(4):
                nc.tensor.transpose(
                    vt_ps[:, g, :],
                    v_sb[:, p0 + 4 * g:p0 + 4 * g + 4, :].opt(),
                    identity,
                )
            exp_sb = asb.tile([128, 4, 128], BF16, tag="exp")
            nc.scalar.activation(
                exp_sb, scores, mybir.ActivationFunctionType.Exp, scale=inv_sqrt_d
            )
            masked = asb.tile([128, 4, 128], BF16, tag="mk")
            nc.vector.tensor_tensor(
                masked,
                exp_sb,
                mask_sb[:, None, :].to_broadcast((128, 4, 128)),
                mybir.AluOpType.mult,
            )
            nc.vector.tensor_reduce(
                zs_sb[:, 4 * blk:4 * blk + 4],
                masked,
                axis=mybir.AxisListType.X,
                op=mybir.AluOpType.add,
            )
            probsT = asb.tile([128, 4, 128], BF16, tag="pt")
            nc.vector.transpose(probsT, masked)  # block-diagonal -> true transpose
            vt_sb = asb.tile([128, 4, 128], BF16, tag="vts")
            nc.scalar.copy(vt_sb, vt_ps)
            return (probsT, vt_sb, sstart, p0)

        def emit_attn_back(pend):
            """PV + ctx output of a previously fronted block."""
            probsT, vt_sb, sstart, p0 = pend
            ctd = cdp.tile([128, 4, 128], F32, tag="ctd")
            for g in range(4):
                nc.tensor.matmul(
                    ctd[:, g, :],
                    lhsT=vt_sb[:, g, :],
                    rhs=probsT[:, g, :],
                    start=True,
                    stop=True,
                )
            ct_blk = ctp.tile([128, 16, H], BF16, tag="ct")
            nc.vector.tensor_scalar(
                out=ct_blk.rearrange("d a b -> d (a b)"),
                in0=ctd.rearrange("d a b -> d (a b)"),
                scalar1=0.0,
                scalar2=None,
                op0=mybir.AluOpType.add,
            )
            nc.sync.dma_start(
                ctx_out[:, sstart + p0:sstart + p0 + 16, :], ct_blk
            )

        def fetch_xt(si):
            s0 = sum(SLABS[:si])
            xt_sb = xtp.tile([128, KT, SLABS[si]], BF16, tag="xt")
            for kc in range(4):
                nc.sync.dma_start(
                    xt_sb[:, 8 * kc:8 * kc + 8, :],
                    xt[:, 8 * kc:8 * kc + 8, s0:s0 + SLABS[si]],
                )
            return xt_sb

        prev_tiles = None
        pending = None
        sstart = 0
        next_xt = fetch_xt(0)
        for si, SL in enumerate(SLABS):
            xt_sb = next_xt
            q_sb = qkvp.tile([128, SL, H], BF16, tag="q")
            k_sb = qkvp.tile([128, SL, H], BF16, tag="k")
            v_sb = qkvp.tile([128, SL, H], BF16, tag="v")
            zs_sb = zsp.tile([128, SL // 4], F32, tag="zs")
            dsts = (q_sb, k_sb, v_sb)

            nblk_prev = SLABS[si - 1] // 16 if si > 0 else 0
            attn_j = 0
            for ft in range(FT):
                w_sb = wp.tile([128, KT, 128], FP8, tag="w")
                wsrc = wt[ft].rearrange("p (a b) -> p a b", a=KT)
                if si == 0 and ft == 0:
                    for kc in range(4):
                        nc.sync.dma_start(
                            w_sb[:, 8 * kc:8 * kc + 8, :], wsrc[:, 8 * kc:8 * kc + 8, :]
                        )
                else:
                    nc.sync.dma_start(w_sb, wsrc)
                ps = psp.tile([128, SL], F32, tag="ps", padded_shape=[128, 512])
                for kt in range(KT):
                    nc.tensor.matmul(
                        ps,
                        lhsT=w_sb[:, kt, :],
                        rhs=xt_sb[:, kt, :],
                        start=(kt == 0),
                        stop=(kt == KT - 1),
                    )
                # bias add (per-partition scalar) + cast to bf16, PSUM -> SBUF
                nc.vector.tensor_scalar(
                    out=dsts[ft // H][:, :, ft % H],
                    in0=ps,
                    scalar1=bias_sb[:, ft:ft + 1],
                    scalar2=None,
                    op0=mybir.AluOpType.add,
                )
                if ft == 8 and si + 1 < len(SLABS):
                    next_xt = fetch_xt(si + 1)
                # interleave previous slab's attention blocks across the ft loop
                while attn_j < nblk_prev * (ft + 1) // FT:
                    front = emit_attn_front(prev_tiles, attn_j)
                    if pending is not None:
                        emit_attn_back(pending)
                    pending = front
                    attn_j += 1
            if si > 0:
                nc.sync.dma_start(
                    zsum_out[:, prev_tiles[4] // 4:(prev_tiles[4] + SLABS[si - 1]) // 4],
                    prev_tiles[3],
                )
            prev_tiles = (q_sb, k_sb, v_sb, zs_sb, sstart)
            sstart += SL

        # last slab's attention (the only non-overlapped part)
        for blk in range(SLABS[-1] // 16):
            front = emit_attn_front(prev_tiles, blk)
            if pending is not None:
                emit_attn_back(pending)
            pending = front
        emit_attn_back(pending)
        nc.sync.dma_start(
            zsum_out[:, prev_tiles[4] // 4:(prev_tiles[4] + SLABS[-1]) // 4],
            prev_tiles[3],
        )

    nc.compile()
    return nc


def get_nc():
    global _CACHED_NC
    if _CACHED_NC is None:
        _CACHED_NC = build_nc()
    return _CACHED_NC


def prep_inputs(hidden_states, wq, bq, wk, bk, wv, bv):
    """Host-side layout prep. Returns per-core input maps."""
    bf16 = ml_dtypes.bfloat16

    # X^T tiled: [ipart, kt, p] with p the global position index
    xt_all = (
        np.ascontiguousarray(hidden_states.reshape(P_TOT, E).T)
        .astype(bf16)
        .reshape(KT, 128, P_TOT)
        .transpose(1, 0, 2)
    )  # [128, KT, 8192] (view)

    # Fused weight W[12288, 4096] -> W^T tiled [ft, ipart, kt*128 + f].
    # Weights are fp8-e4m3 roundtripped, so fp8 storage is lossless.
    wcat = np.concatenate([wq, wk, wv], axis=0)  # [3E, E]
    wt = (
        np.ascontiguousarray(wcat.T)
        .astype(ml_dtypes.float8_e4m3)
        .reshape(KT, 128, FT, 128)
        .transpose(2, 1, 0, 3)
    )
    wt = np.ascontiguousarray(wt).reshape(FT, 128, KT * 128)

    bias_cols = np.ascontiguousarray(
        np.concatenate([bq, bk, bv]).astype(np.float32).reshape(FT, 128).T
    )  # [128, FT]

    mask = np.zeros((128, 128), dtype=bf16)
    for p in range(4):
        mask[32 * p:32 * p + 32, 32 * p:32 * p + 32] = 1.0

    in_maps = []
    for c in range(N_CORES):
        xt_c = np.ascontiguousarray(xt_all[:, :, c * P_CORE:(c + 1) * P_CORE])
        in_maps.append({"xt": xt_c, "wt": wt, "bias": bias_cols, "maskd": mask})
    return in_maps


def normalize_shard(ctx_u, zsum):
    """ctx_u [128, P_CORE, H] bf16 (d, pos, h) unnormalized; zsum [128, P_CORE//4].

    Returns normalized [P_CORE, E] fp32. z for position pos, head h lives at
    zsum[32*(pos%4)+h, pos//4]."""
    ctx = np.asarray(ctx_u).astype(np.float32).transpose(1, 2, 0)  # [pos, h, d]
    z = np.asarray(zsum).astype(np.float32)  # [128, P_CORE//4]
    z = z.reshape(4, 32, P_CORE // 4).transpose(2, 0, 1).reshape(P_CORE, 32)
    return (ctx / z[:, :, None]).reshape(P_CORE, E)


def assemble_output(ctxs, zsums):
    out = np.empty((P_TOT, E), dtype=np.float32)
    for c in range(N_CORES):
        out[c * P_CORE:(c + 1) * P_CORE] = normalize_shard(ctxs[c], zsums[c])
    return out.reshape(B, S, E)


def kernel(**inputs):
    from concourse.bass_utils import run_bass_kernel_spmd

    nc = get_nc()
    in_maps = prep_inputs(
        inputs["hidden_states"],
        inputs["wq"], inputs["bq"],
        inputs["wk"], inputs["bk"],
        inputs["wv"], inputs["bv"],
    )
    res = run_bass_kernel_spmd(nc, in_maps, core_ids=list(range(N_CORES)))
    ctxs = [np.asarray(r["ctx"]).reshape(128, P_CORE, H) for r in res.results]
    zsums = [np.asarray(r["zsum"]).reshape(128, P_CORE // 4) for r in res.results]
    return assemble_output(ctxs, zsums)


# revision 26
# speedup vs baseline: 1.3313x; 1.0139x over previous
"""Trainium2 Bass kernel for nn_MultiHeadLatentAttention_82068235092052.

Reference computation (B=2, S=4096, E=4096, H=32, D=128):
    q = hs @ wq.T + bq   -> [B,S,H,D]     (wq/bq are fp8-roundtripped fp32)
    k = hs @ wk.T + bk
    v = hs @ wv.T + bv
    (latent = hs @ wl.T + bl is computed but UNUSED -> skipped entirely)
    scores  = einsum('bshd,bstd->bsht', q, k) / sqrt(D)   # attention over HEADS per position
    probs   = softmax(scores, -1)
    context = einsum('bsht,bstd->bshd', probs, v).reshape(B,S,E)

Strategy: data-parallel over the 8192 positions across 8 cores (1024 each,
processed in 2 halves of 512). Per core, one fused bf16 matmul
W[12288,4096] x X^T produces q/k/v in feature-major layout [d, head, pos]
(each 128-row feature tile == one head), which feeds per-position 32x32
head-attention done with tile_position-packed PE matmuls + PE transposes.
Softmax normalization is a per-partition tensor_scalar; 1/sqrt(D) is folded
into the exp activation's scale.

Weights are exactly representable in bf16 (fp8 e4m3 subset), so the only
quantization error is the activations' fp32->bf16 rounding.
"""

import os
import sys

import numpy as np

sys.path.insert(0, "/opt/trn_rl_repo")

import ml_dtypes

import concourse.bacc as bacc
import concourse.bass as bass
import concourse.tile as tile
from concourse import mybir
from concourse.masks import make_identity

# Problem constants (hardcoded; kernel.py must be self-contained).
B, S, E = 2, 4096, 4096
H, D = 32, 128
P_TOT = B * S            # 8192 positions
N_CORES = 8
P_CORE = P_TOT // N_CORES  # 1024 positions per core
HALF = P_CORE // 2         # 512 positions per half
FT = 3 * H                 # 96 feature tiles (q, k, v concatenated)
KT = E // 128              # 32 contraction tiles

BF16 = mybir.dt.bfloat16
F32 = mybir.dt.float32
FP8 = mybir.dt.float8e4

_CACHED_NC = None


def build_nc():
    """Build the per-core Bass program (same program on all 8 cores)."""
    nc = bacc.Bacc(
        "TRN2",
        target_bir_lowering=False,
        debug=False,
        enable_asserts=True,
        num_devices=1,
    )

    xt = nc.dram_tensor("xt", [128, KT, P_CORE], BF16, kind="ExternalInput").ap()
    wt = nc.dram_tensor("wt", [FT, 128, KT * 128], FP8, kind="ExternalInput").ap()
    bias = nc.dram_tensor("bias", [128, FT], F32, kind="ExternalInput").ap()
    ctx_out = nc.dram_tensor("ctx", [128, P_CORE, H], BF16, kind="ExternalOutput").ap()

    from contextlib import ExitStack

    with tile.TileContext(nc) as tc, ExitStack() as stack:
        const = stack.enter_context(tc.tile_pool(name="const", bufs=1))
        xtp = stack.enter_context(tc.tile_pool(name="xtp", bufs=1))
        qkvp = stack.enter_context(tc.tile_pool(name="qkvp", bufs=1))
        wp = stack.enter_context(tc.tile_pool(name="wp", bufs=2))
        ctp = stack.enter_context(tc.tile_pool(name="ctp", bufs=1))
        ap_pool = stack.enter_context(tc.tile_pool(name="attn", bufs=3))
        psum = stack.enter_context(tc.tile_pool(name="psum", bufs=2, space="PSUM"))
        sc_pool = stack.enter_context(tc.tile_pool(name="scps", bufs=1, space="PSUM"))
        vt_pool = stack.enter_context(tc.tile_pool(name="vtps", bufs=2, space="PSUM"))
        ct_pool = stack.enter_context(tc.tile_pool(name="ctps", bufs=2, space="PSUM"))

        identity = const.tile([128, 128], BF16)
        make_identity(nc, identity)
        bias_sb = const.tile([128, FT], F32)
        nc.sync.dma_start(bias_sb, bias)

        inv_sqrt_d = 1.0 / float(np.sqrt(D))

        # two persistent block-diagonal score banks: off-diagonal -1e30 is
        # written once here and survives (QK only overwrites the diagonals)
        score_tiles = []
        for i in range(2):
            sct = sc_pool.tile([128, 4, 128], F32, tag=f"sc{i}")
            nc.vector.memset(sct, -1e30)
            score_tiles.append(sct)
        blk_counter = [0]

        for hf in range(2):
            # ---- projections: qkv[d, ft, p] = sum_i W[ft*128+d, i] * X[p, i] (+ bias)
            xt_sb = xtp.tile([128, KT, HALF], BF16, tag="xt")
            for kc in range(4):
                nc.sync.dma_start(
                    xt_sb[:, 8 * kc:8 * kc + 8, :],
                    xt[:, 8 * kc:8 * kc + 8, hf * HALF:(hf + 1) * HALF],
                )
            qk_sb = qkvp.tile([128, 2 * H, HALF], BF16, tag="qk")
            v_sb = qkvp.tile([128, HALF, H], BF16, tag="v")

            for ft in range(FT):
                w_sb = wp.tile([128, KT, 128], FP8, tag="w")
                nc.sync.dma_start(
                    w_sb, wt[ft].rearrange("p (a b) -> p a b", a=KT)
                )
                ps = psum.tile([128, HALF], F32, tag="ps")
                for kt in range(KT):
                    nc.tensor.matmul(
                        ps,
                        lhsT=w_sb[:, kt, :],
                        rhs=xt_sb[:, kt, :],
                        start=(kt == 0),
                        stop=(kt == KT - 1),
                    )
                # bias add (per-partition scalar) + cast to bf16, PSUM -> SBUF
                if ft < 2 * H:
                    dst = qk_sb[:, ft, :]
                else:
                    dst = v_sb[:, :, ft - 2 * H]
                nc.vector.tensor_scalar(
                    out=dst,
                    in0=ps,
                    scalar1=bias_sb[:, ft:ft + 1],
                    scalar2=None,
                    op0=mybir.AluOpType.add,
                )

            # ---- attention: software-pipelined blocks of 16 positions.
            # PV(b-1) is emitted AFTER QK/VT(b) so the in-order PE stream has
            # block b's independent work to run while b-1's DVE chain finishes.
            ct_sb = ctp.tile([128, HALF, H], BF16, tag="ct")
            pending = None  # (probsT, vt_sb, p0) awaiting PV
            def emit_pv(pend):
                probsT_p, vt_sb_p, p0_p = pend
                ctd = ct_pool.tile([128, 4, 128], F32, tag="ctd")
                for g in range(4):
                    nc.tensor.matmul(
                        ctd[:, g, :],
                        lhsT=vt_sb_p[:, g, :],
                        rhs=probsT_p[:, g, :],
                        start=True,
                        stop=True,
                    )
                nc.scalar.copy(ct_sb[:, p0_p:p0_p + 16, :], ctd)
            for blk in range(HALF // 16):
                p0 = blk * 16
                scores = score_tiles[blk_counter[0] % 2]
                blk_counter[0] += 1
                for g in range(4):
                    for j in range(4):
                        nc.tensor.matmul(
                            scores[32 * j:32 * j + 32, g, 32 * j:32 * j + 32],
                            lhsT=qk_sb[:, 0:H, p0 + 4 * g + j],
                            rhs=qk_sb[:, H:2 * H, p0 + 4 * g + j],
                            start=True,
                            stop=True,
                            tile_position=(0, 32 * j),
                        )
                vt_ps = vt_pool.tile([128, 4, 128], BF16, tag="vt")
                for g in range(4):
                    nc.tensor.transpose(
                        vt_ps[:, g, :],
                        v_sb[:, p0 + 4 * g:p0 + 4 * g + 4, :].opt(),
                        identity,
                    )
                exp_sb = ap_pool.tile([128, 4, 128], BF16, tag="exp")
                nc.scalar.activation(
                    exp_sb,
                    scores,
                    mybir.ActivationFunctionType.Exp,
                    scale=inv_sqrt_d,
                )
                zsum = ap_pool.tile([128, 4], F32, tag="z")
                nc.vector.tensor_reduce(
                    zsum, exp_sb, axis=mybir.AxisListType.X, op=mybir.AluOpType.add
                )
                zinv = ap_pool.tile([128, 4], F32, tag="zi")
                nc.vector.reciprocal(zinv, zsum)
                probs = ap_pool.tile([128, 4, 128], BF16, tag="pb")
                nc.vector.tensor_tensor(
                    probs,
                    exp_sb,
                    zinv[:, :, None].to_broadcast((128, 4, 128)),
                    mybir.AluOpType.mult,
                )
                probsT = ap_pool.tile([128, 4, 128], BF16, tag="pt")
                nc.vector.transpose(probsT, probs)
                vt_sb = ap_pool.tile([128, 4, 128], BF16, tag="vts")
                nc.scalar.copy(vt_sb, vt_ps)
                if pending is not None:
                    emit_pv(pending)
                pending = (probsT, vt_sb, p0)
            emit_pv(pending)
            nc.sync.dma_start(ctx_out[:, hf * HALF:(hf + 1) * HALF, :], ct_sb)

    nc.compile()
    return nc


def get_nc():
    global _CACHED_NC
    if _CACHED_NC is None:
        _CACHED_NC = build_nc()
    return _CACHED_NC


def prep_inputs(hidden_states, wq, bq, wk, bk, wv, bv):
    """Host-side layout prep. Returns (in_maps, None)."""
    bf16 = ml_dtypes.bfloat16

    # X^T tiled: [ipart, kt, p] with p the global position index
    xt_all = (
        np.ascontiguousarray(hidden_states.reshape(P_TOT, E).T)
        .astype(bf16)
        .reshape(KT, 128, P_TOT)
        .transpose(1, 0, 2)
    )  # [128, KT, 8192] (view)

    # Fused weight W[12288, 4096] -> W^T tiled [ft, ipart, kt*128 + f]
    # Weights are fp8-e4m3 roundtripped, so fp8 storage is lossless.
    wcat = np.concatenate([wq, wk, wv], axis=0)  # [3E, E]
    wt = (
        np.ascontiguousarray(wcat.T)
        .astype(ml_dtypes.float8_e4m3)
        .reshape(KT, 128, FT, 128)
        .transpose(2, 1, 0, 3)
    )
    wt = np.ascontiguousarray(wt).reshape(FT, 128, KT * 128)

    bias_cols = np.ascontiguousarray(
        np.concatenate([bq, bk, bv]).astype(np.float32).reshape(FT, 128).T
    )  # [128, FT]

    in_maps = []
    for c in range(N_CORES):
        xt_c = np.ascontiguousarray(xt_all[:, :, c * P_CORE:(c + 1) * P_CORE])
        in_maps.append({"xt": xt_c, "wt": wt, "bias": bias_cols})
    return in_maps


def assemble_output(ctx_per_core):
    """ctx_per_core: list of [128, H, P_CORE] bf16 arrays -> [B, S, E] fp32."""
    full = np.concatenate(ctx_per_core, axis=1)  # [d=128, p=8192, h=32]
    out = full.transpose(1, 2, 0).astype(np.float32)  # [p, h, d]
    return np.ascontiguousarray(out.reshape(B, S, E))


def kernel(**inputs):
    from concourse.bass_utils import run_bass_kernel_spmd

    nc = get_nc()
    in_maps = prep_inputs(
        inputs["hidden_states"],
        inputs["wq"], inputs["bq"],
        inputs["wk"], inputs["bk"],
        inputs["wv"], inputs["bv"],
    )
    res = run_bass_kernel_spmd(nc, in_maps, core_ids=list(range(N_CORES)))
    ctxs = [np.asarray(r["ctx"]).reshape(128, P_CORE, H) for r in res.results]
    return assemble_output(ctxs)



# revision 27
# speedup vs baseline: 1.4186x; 1.0656x over previous
"""Trainium2 Bass kernel for nn_MultiHeadLatentAttention_82068235092052.

Reference computation (B=2, S=4096, E=4096, H=32, D=128):
    q = hs @ wq.T + bq   -> [B,S,H,D]     (wq/bq are fp8-roundtripped fp32)
    k = hs @ wk.T + bk
    v = hs @ wv.T + bv
    (latent = hs @ wl.T + bl is computed but UNUSED -> skipped entirely)
    scores  = einsum('bshd,bstd->bsht', q, k) / sqrt(D)   # attention over HEADS per position
    probs   = softmax(scores, -1)
    context = einsum('bsht,bstd->bshd', probs, v).reshape(B,S,E)

Strategy: data-parallel over the 8192 positions across 8 cores (1024 each).
Per core the positions are processed in 5 slabs (256/256/256/192/64); the
per-position 32x32 head-attention of slab s-1 is interleaved into the
projection matmuls of slab s, so only the tiny last slab's attention is
exposed at the end.

Projections stream the fused W[12288,4096] weights as fp8-e4m3 (lossless:
the reference weights are fp8-roundtripped) into mixed fp8xbf16 matmuls.
q/k/v land pos-major [d, pos, head]; per 16-position block the scores are
computed with 4 cross-position matmuls (N=128, 4 positions each), the
off-diagonal cross terms are killed by a block-diagonal mask fused into the
tensor_tensor_reduce that also produces the softmax denominators (zsum).
probs stay UNNORMALIZED on device; zsum ships to the host, which divides in
fp32 while assembling the output.
"""

import os
import sys

import numpy as np

sys.path.insert(0, "/opt/trn_rl_repo")

import ml_dtypes

import concourse.bacc as bacc
import concourse.bass as bass
import concourse.tile as tile
from concourse import mybir
from concourse.masks import make_identity

# Problem constants (hardcoded; kernel.py must be self-contained).
B, S, E = 2, 4096, 4096
H, D = 32, 128
P_TOT = B * S            # 8192 positions
N_CORES = 8
P_CORE = P_TOT // N_CORES  # 1024 positions per core
FT = 3 * H                 # 96 feature tiles (q, k, v concatenated)
KT = E // 128              # 32 contraction tiles

SLABS = [288, 288, 288, 160]
assert sum(SLABS) == P_CORE and all(s % 16 == 0 for s in SLABS)
SMAX = max(SLABS)

BF16 = mybir.dt.bfloat16
F32 = mybir.dt.float32
FP8 = mybir.dt.float8e4

_CACHED_NC = None


def build_nc():
    """Build the per-core Bass program (same program on all 8 cores)."""
    nc = bacc.Bacc(
        "TRN2",
        target_bir_lowering=False,
        debug=False,
        enable_asserts=True,
        num_devices=1,
    )

    xt = nc.dram_tensor("xt", [128, KT, P_CORE], BF16, kind="ExternalInput").ap()
    wt = nc.dram_tensor("wt", [FT, 128, KT * 128], FP8, kind="ExternalInput").ap()
    bias = nc.dram_tensor("bias", [128, FT], F32, kind="ExternalInput").ap()
    maskd = nc.dram_tensor("maskd", [128, 128], BF16, kind="ExternalInput").ap()
    ctx_out = nc.dram_tensor("ctx", [128, P_CORE, H], BF16, kind="ExternalOutput").ap()
    zsum_out = nc.dram_tensor("zsum", [128, P_CORE // 4], F32, kind="ExternalOutput").ap()

    from contextlib import ExitStack

    with tile.TileContext(nc) as tc, ExitStack() as stack:
        const = stack.enter_context(tc.tile_pool(name="const", bufs=1))
        xtp = stack.enter_context(tc.tile_pool(name="xtp", bufs=2))
        qkvp = stack.enter_context(tc.tile_pool(name="qkvp", bufs=2))
        wp = stack.enter_context(tc.tile_pool(name="wp", bufs=3))
        zsp = stack.enter_context(tc.tile_pool(name="zsp", bufs=2))
        asb = stack.enter_context(tc.tile_pool(name="asb", bufs=2))
        ctp = stack.enter_context(tc.tile_pool(name="ctp", bufs=3))
        psp = stack.enter_context(tc.tile_pool(name="psp", bufs=2, space="PSUM"))
        scp = stack.enter_context(tc.tile_pool(name="scp", bufs=2, space="PSUM"))
        vtp = stack.enter_context(tc.tile_pool(name="vtp", bufs=2, space="PSUM"))
        cdp = stack.enter_context(tc.tile_pool(name="cdp", bufs=2, space="PSUM"))

        identity = const.tile([128, 128], BF16)
        make_identity(nc, identity)
        bias_sb = const.tile([128, FT], F32)
        nc.sync.dma_start(bias_sb, bias)
        mask_sb = const.tile([128, 128], BF16)
        nc.sync.dma_start(mask_sb, maskd)

        inv_sqrt_d = 1.0 / float(np.sqrt(D))

        def emit_attn_front(slab_tiles, blk):
            """QK + VT + softmax front half of one 16-position block."""
            q_sb, k_sb, v_sb, zs_sb, sstart = slab_tiles
            p0 = blk * 16
            scores = scp.tile([128, 4, 128], F32, tag="sc")
            for g in range(4):
                nc.tensor.matmul(
                    scores[:, g, :],
                    lhsT=q_sb[:, p0 + 4 * g:p0 + 4 * g + 4, :],
                    rhs=k_sb[:, p0 + 4 * g:p0 + 4 * g + 4, :],
                    start=True,
                    stop=True,
                )
            vt_ps = vtp.tile([128, 4, 128], BF16, tag="vt", padded_shape=[128, 4, 256])
            for g in range(4):
                nc.tensor.transpose(
                    vt_ps[:, g, :],
                    v_sb[:, p0 + 4 * g:p0 + 4 * g + 4, :].opt(),
                    identity,
                )
            exp_sb = asb.tile([128, 4, 128], BF16, tag="exp")
            nc.scalar.activation(
                exp_sb, scores, mybir.ActivationFunctionType.Exp, scale=inv_sqrt_d
            )
            masked = asb.tile([128, 4, 128], BF16, tag="mk")
            nc.vector.tensor_tensor(
                masked,
                exp_sb,
                mask_sb[:, None, :].to_broadcast((128, 4, 128)),
                mybir.AluOpType.mult,
            )
            nc.vector.tensor_reduce(
                zs_sb[:, 4 * blk:4 * blk + 4],
                masked,
                axis=mybir.AxisListType.X,
                op=mybir.AluOpType.add,
            )
            probsT = asb.tile([128, 4, 128], BF16, tag="pt")
            nc.vector.transpose(probsT, masked)  # block-diagonal -> true transpose
            vt_sb = asb.tile([128, 4, 128], BF16, tag="vts")
            nc.scalar.copy(vt_sb, vt_ps)
            return (probsT, vt_sb, sstart, p0)

        def emit_attn_back(pend):
            """PV + ctx output of a previously fronted block."""
            probsT, vt_sb, sstart, p0 = pend
            ctd = cdp.tile([128, 4, 128], F32, tag="ctd")
            for g in range(4):
                nc.tensor.matmul(
                    ctd[:, g, :],
                    lhsT=vt_sb[:, g, :],
                    rhs=probsT[:, g, :],
                    start=True,
                    stop=True,
                )
            ct_blk = ctp.tile([128, 16, H], BF16, tag="ct")
            nc.vector.tensor_scalar(
                out=ct_blk.rearrange("d a b -> d (a b)"),
                in0=ctd.rearrange("d a b -> d (a b)"),
                scalar1=0.0,
                scalar2=None,
                op0=mybir.AluOpType.add,
            )
            nc.sync.dma_start(
                ctx_out[:, sstart + p0:sstart + p0 + 16, :], ct_blk
            )

        def fetch_xt(si):
            s0 = sum(SLABS[:si])
            xt_sb = xtp.tile([128, KT, SLABS[si]], BF16, tag="xt")
            for kc in range(4):
                nc.sync.dma_start(
                    xt_sb[:, 8 * kc:8 * kc + 8, :],
                    xt[:, 8 * kc:8 * kc + 8, s0:s0 + SLABS[si]],
                )
            return xt_sb

        prev_tiles = None
        pending = None
        sstart = 0
        next_xt = fetch_xt(0)
        for si, SL in enumerate(SLABS):
            xt_sb = next_xt
            q_sb = qkvp.tile([128, SL, H], BF16, tag="q")
            k_sb = qkvp.tile([128, SL, H], BF16, tag="k")
            v_sb = qkvp.tile([128, SL, H], BF16, tag="v")
            zs_sb = zsp.tile([128, SL // 4], F32, tag="zs")
            dsts = (q_sb, k_sb, v_sb)

            nblk_prev = SLABS[si - 1] // 16 if si > 0 else 0
            attn_j = 0
            for ft in range(FT):
                w_sb = wp.tile([128, KT, 128], FP8, tag="w")
                wsrc = wt[ft].rearrange("p (a b) -> p a b", a=KT)
                if si == 0 and ft == 0:
                    for kc in range(4):
                        nc.sync.dma_start(
                            w_sb[:, 8 * kc:8 * kc + 8, :], wsrc[:, 8 * kc:8 * kc + 8, :]
                        )
                else:
                    nc.sync.dma_start(w_sb, wsrc)
                ps = psp.tile([128, SL], F32, tag="ps", padded_shape=[128, 512])
                for kt in range(KT):
                    nc.tensor.matmul(
                        ps,
                        lhsT=w_sb[:, kt, :],
                        rhs=xt_sb[:, kt, :],
                        start=(kt == 0),
                        stop=(kt == KT - 1),
                    )
                # bias add (per-partition scalar) + cast to bf16, PSUM -> SBUF
                nc.vector.tensor_scalar(
                    out=dsts[ft // H][:, :, ft % H],
                    in0=ps,
                    scalar1=bias_sb[:, ft:ft + 1],
                    scalar2=None,
                    op0=mybir.AluOpType.add,
                )
                if ft == 8 and si + 1 < len(SLABS):
                    next_xt = fetch_xt(si + 1)
                # interleave previous slab's attention blocks across the ft loop
                while attn_j < nblk_prev * (ft + 1) // FT:
                    front = emit_attn_front(prev_tiles, attn_j)
                    if pending is not None:
                        emit_attn_back(pending)
                    pending = front
                    attn_j += 1
            if si > 0:
                nc.sync.dma_start(
                    zsum_out[:, prev_tiles[4] // 4:(prev_tiles[4] + SLABS[si - 1]) // 4],
                    prev_tiles[3],
                )
            prev_tiles = (q_sb, k_sb, v_sb, zs_sb, sstart)
            sstart += SL

        # last slab's attention (the only non-overlapped part)
        for blk in range(SLABS[-1] // 16):
            front = emit_attn_front(prev_tiles, blk)
            if pending is not None:
                emit_attn_back(pending)
            pending = front
        emit_attn_back(pending)
        nc.sync.dma_start(
            zsum_out[:, prev_tiles[4] // 4:(prev_tiles[4] + SLABS[-1]) // 4],
            prev_tiles[3],
        )

    nc.compile()
    return nc


def get_nc():
    global _CACHED_NC
    if _CACHED_NC is None:
        _CACHED_NC = build_nc()
    return _CACHED_NC


def prep_inputs(hidden_states, wq, bq, wk, bk, wv, bv):
    """Host-side layout prep. Returns per-core input maps."""
    bf16 = ml_dtypes.bfloat16

    # X^T tiled: [ipart, kt, p] with p the global position index
    xt_all = (
        np.ascontiguousarray(hidden_states.reshape(P_TOT, E).T)
        .astype(bf16)
        .reshape(KT, 128, P_TOT)
        .transpose(1, 0, 2)
    )  # [128, KT, 8192] (view)

    # Fused weight W[12288, 4096] -> W^T tiled [ft, ipart, kt*128 + f].
    # Weights are fp8-e4m3 roundtripped, so fp8 storage is lossless.
    wcat = np.concatenate([wq, wk, wv], axis=0)  # [3E, E]
    wt = (
        np.ascontiguousarray(wcat.T)
        .astype(ml_dtypes.float8_e4m3)
        .reshape(KT, 128, FT, 128)
        .transpose(2, 1, 0, 3)
    )
    wt = np.ascontiguousarray(wt).reshape(FT, 128, KT * 128)

    bias_cols = np.ascontiguousarray(
        np.concatenate([bq, bk, bv]).astype(np.float32).reshape(FT, 128).T
    )  # [128, FT]

    mask = np.zeros((128, 128), dtype=bf16)
    for p in range(4):
        mask[32 * p:32 * p + 32, 32 * p:32 * p + 32] = 1.0

    in_maps = []
    for c in range(N_CORES):
        xt_c = np.ascontiguousarray(xt_all[:, :, c * P_CORE:(c + 1) * P_CORE])
        in_maps.append({"xt": xt_c, "wt": wt, "bias": bias_cols, "maskd": mask})
    return in_maps


def normalize_shard(ctx_u, zsum):
    """ctx_u [128, P_CORE, H] bf16 (d, pos, h) unnormalized; zsum [128, P_CORE//4].

    Returns normalized [P_CORE, E] fp32. z for position pos, head h lives at
    zsum[32*(pos%4)+h, pos//4]."""
    ctx = np.asarray(ctx_u).astype(np.float32).transpose(1, 2, 0)  # [pos, h, d]
    z = np.asarray(zsum).astype(np.float32)  # [128, P_CORE//4]
    z = z.reshape(4, 32, P_CORE // 4).transpose(2, 0, 1).reshape(P_CORE, 32)
    return (ctx / z[:, :, None]).reshape(P_CORE, E)


def assemble_output(ctxs, zsums):
    out = np.empty((P_TOT, E), dtype=np.float32)
    for c in range(N_CORES):
        out[c * P_CORE:(c + 1) * P_CORE] = normalize_shard(ctxs[c], zsums[c])
    return out.reshape(B, S, E)


def kernel(**inputs):
    from concourse.bass_utils import run_bass_kernel_spmd

    nc = get_nc()
    in_maps = prep_inputs(
        inputs["hidden_states"],
        inputs["wq"], inputs["bq"],
        inputs["wk"], inputs["bk"],
        inputs["wv"], inputs["bv"],
    )
    res = run_bass_kernel_spmd(nc, in_maps, core_ids=list(range(N_CORES)))
    ctxs = [np.asarray(r["ctx"]).reshape(128, P_CORE, H) for r in res.results]
    zsums = [np.asarray(r["zsum"]).reshape(128, P_CORE // 4) for r in res.results]
    return assemble_output(ctxs, zsums)


# revision 29
# speedup vs baseline: 1.4217x; 1.0022x over previous
"""Trainium2 Bass kernel for nn_MultiHeadLatentAttention_82068235092052.

Reference computation (B=2, S=4096, E=4096, H=32, D=128):
    q = hs @ wq.T + bq   -> [B,S,H,D]     (wq/bq are fp8-roundtripped fp32)
    k = hs @ wk.T + bk
    v = hs @ wv.T + bv
    (latent = hs @ wl.T + bl is computed but UNUSED -> skipped entirely)
    scores  = einsum('bshd,bstd->bsht', q, k) / sqrt(D)   # attention over HEADS per position
    probs   = softmax(scores, -1)
    context = einsum('bsht,bstd->bshd', probs, v).reshape(B,S,E)

Strategy: data-parallel over the 8192 positions across 8 cores (1024 each).
Per core the positions are processed in 5 slabs (256/256/256/192/64); the
per-position 32x32 head-attention of slab s-1 is interleaved into the
projection matmuls of slab s, so only the tiny last slab's attention is
exposed at the end.

Projections stream the fused W[12288,4096] weights as fp8-e4m3 (lossless:
the reference weights are fp8-roundtripped) into mixed fp8xbf16 matmuls.
q/k/v land pos-major [d, pos, head]; per 16-position block the scores are
computed with 4 cross-position matmuls (N=128, 4 positions each), the
off-diagonal cross terms are killed by a block-diagonal mask fused into the
tensor_tensor_reduce that also produces the softmax denominators (zsum).
probs stay UNNORMALIZED on device; zsum ships to the host, which divides in
fp32 while assembling the output.
"""

import os
import sys

import numpy as np

sys.path.insert(0, "/opt/trn_rl_repo")

import ml_dtypes

import concourse.bacc as bacc
import concourse.bass as bass
import concourse.tile as tile
from concourse import mybir
from concourse.masks import make_identity

# Problem constants (hardcoded; kernel.py must be self-contained).
B, S, E = 2, 4096, 4096
H, D = 32, 128
P_TOT = B * S            # 8192 positions
N_CORES = 8
P_CORE = P_TOT // N_CORES  # 1024 positions per core
FT = 3 * H                 # 96 feature tiles (q, k, v concatenated)
KT = E // 128              # 32 contraction tiles

SLABS = [288, 288, 288, 160]
assert sum(SLABS) == P_CORE and all(s % 16 == 0 for s in SLABS)
SMAX = max(SLABS)

BF16 = mybir.dt.bfloat16
F32 = mybir.dt.float32
FP8 = mybir.dt.float8e4

_CACHED_NC = None


def build_nc():
    """Build the per-core Bass program (same program on all 8 cores)."""
    nc = bacc.Bacc(
        "TRN2",
        target_bir_lowering=False,
        debug=False,
        enable_asserts=True,
        num_devices=1,
    )

    xt = nc.dram_tensor("xt", [128, KT, P_CORE], BF16, kind="ExternalInput").ap()
    wt = nc.dram_tensor("wt", [FT, 128, KT * 128], FP8, kind="ExternalInput").ap()
    bias = nc.dram_tensor("bias", [128, FT], F32, kind="ExternalInput").ap()
    maskd = nc.dram_tensor("maskd", [128, 128], BF16, kind="ExternalInput").ap()
    ctx_out = nc.dram_tensor("ctx", [128, P_CORE, H], BF16, kind="ExternalOutput").ap()
    zsum_out = nc.dram_tensor("zsum", [128, P_CORE // 4], F32, kind="ExternalOutput").ap()

    from contextlib import ExitStack

    with tile.TileContext(nc) as tc, ExitStack() as stack:
        const = stack.enter_context(tc.tile_pool(name="const", bufs=1))
        xtp = stack.enter_context(tc.tile_pool(name="xtp", bufs=2))
        qkvp = stack.enter_context(tc.tile_pool(name="qkvp", bufs=2))
        wp = stack.enter_context(tc.tile_pool(name="wp", bufs=3))
        zsp = stack.enter_context(tc.tile_pool(name="zsp", bufs=2))
        asb = stack.enter_context(tc.tile_pool(name="asb", bufs=2))
        ctp = stack.enter_context(tc.tile_pool(name="ctp", bufs=3))
        psp = stack.enter_context(tc.tile_pool(name="psp", bufs=2, space="PSUM"))
        scp = stack.enter_context(tc.tile_pool(name="scp", bufs=2, space="PSUM"))
        vtp = stack.enter_context(tc.tile_pool(name="vtp", bufs=2, space="PSUM"))
        cdp = stack.enter_context(tc.tile_pool(name="cdp", bufs=2, space="PSUM"))

        identity = const.tile([128, 128], BF16)
        make_identity(nc, identity)
        bias_sb = const.tile([128, FT], F32)
        nc.sync.dma_start(bias_sb, bias)
        mask_sb = const.tile([128, 128], BF16)
        nc.sync.dma_start(mask_sb, maskd)

        inv_sqrt_d = 1.0 / float(np.sqrt(D))

        def emit_attn_front(slab_tiles, blk):
            """QK + VT + softmax front half of one 16-position block."""
            q_sb, k_sb, v_sb, zs_sb, sstart = slab_tiles
            p0 = blk * 16
            scores = scp.tile([128, 4, 128], F32, tag="sc")
            for g in range(4):
                nc.tensor.matmul(
                    scores[:, g, :],
                    lhsT=q_sb[:, p0 + 4 * g:p0 + 4 * g + 4, :],
                    rhs=k_sb[:, p0 + 4 * g:p0 + 4 * g + 4, :],
                    start=True,
                    stop=True,
                )
            vt_ps = vtp.tile([128, 4, 128], BF16, tag="vt", padded_shape=[128, 4, 256])
            for g in range(4):
                nc.tensor.transpose(
                    vt_ps[:, g, :],
                    v_sb[:, p0 + 4 * g:p0 + 4 * g + 4, :].opt(),
                    identity,
                )
            exp_sb = asb.tile([128, 4, 128], BF16, tag="exp")
            nc.scalar.activation(
                exp_sb, scores, mybir.ActivationFunctionType.Exp, scale=inv_sqrt_d
            )
            masked = asb.tile([128, 4, 128], BF16, tag="mk")
            nc.vector.tensor_tensor(
                masked,
                exp_sb,
                mask_sb[:, None, :].to_broadcast((128, 4, 128)),
                mybir.AluOpType.mult,
            )
            nc.vector.tensor_reduce(
                zs_sb[:, 4 * blk:4 * blk + 4],
                masked,
                axis=mybir.AxisListType.X,
                op=mybir.AluOpType.add,
            )
            probsT = asb.tile([128, 4, 128], BF16, tag="pt")
            nc.vector.transpose(probsT, masked)  # block-diagonal -> true transpose
            vt_sb = asb.tile([128, 4, 128], BF16, tag="vts")
            nc.scalar.copy(vt_sb, vt_ps)
            return (probsT, vt_sb, sstart, p0)

        def emit_attn_back(pend):
            """PV + ctx output of a previously fronted block."""
            probsT, vt_sb, sstart, p0 = pend
            ctd = cdp.tile([128, 4, 128], F32, tag="ctd")
            for g in range(4):
                nc.tensor.matmul(
                    ctd[:, g, :],
                    lhsT=vt_sb[:, g, :],
                    rhs=probsT[:, g, :],
                    start=True,
                    stop=True,
                )
            ct_blk = ctp.tile([128, 16, H], BF16, tag="ct")
            nc.vector.tensor_scalar(
                out=ct_blk.rearrange("d a b -> d (a b)"),
                in0=ctd.rearrange("d a b -> d (a b)"),
                scalar1=0.0,
                scalar2=None,
                op0=mybir.AluOpType.add,
            )
            nc.sync.dma_start(
                ctx_out[:, sstart + p0:sstart + p0 + 16, :], ct_blk
            )

        def fetch_xt(si):
            s0 = sum(SLABS[:si])
            xt_sb = xtp.tile([128, KT, SLABS[si]], BF16, tag="xt")
            nch = 8 if si == 0 else 4
            w = KT // nch
            for kc in range(nch):
                nc.sync.dma_start(
                    xt_sb[:, w * kc:w * kc + w, :],
                    xt[:, w * kc:w * kc + w, s0:s0 + SLABS[si]],
                )
            return xt_sb

        prev_tiles = None
        pending = None
        sstart = 0
        next_xt = fetch_xt(0)
        for si, SL in enumerate(SLABS):
            xt_sb = next_xt
            q_sb = qkvp.tile([128, SL, H], BF16, tag="q")
            k_sb = qkvp.tile([128, SL, H], BF16, tag="k")
            v_sb = qkvp.tile([128, SL, H], BF16, tag="v")
            zs_sb = zsp.tile([128, SL // 4], F32, tag="zs")
            dsts = (q_sb, k_sb, v_sb)

            nblk_prev = SLABS[si - 1] // 16 if si > 0 else 0
            attn_j = 0
            for ft in range(FT):
                w_sb = wp.tile([128, KT, 128], FP8, tag="w")
                wsrc = wt[ft].rearrange("p (a b) -> p a b", a=KT)
                if si == 0 and ft == 0:
                    for kc in range(4):
                        nc.sync.dma_start(
                            w_sb[:, 8 * kc:8 * kc + 8, :], wsrc[:, 8 * kc:8 * kc + 8, :]
                        )
                else:
                    nc.sync.dma_start(w_sb, wsrc)
                ps = psp.tile([128, SL], F32, tag="ps", padded_shape=[128, 512])
                for kt in range(KT):
                    nc.tensor.matmul(
                        ps,
                        lhsT=w_sb[:, kt, :],
                        rhs=xt_sb[:, kt, :],
                        start=(kt == 0),
                        stop=(kt == KT - 1),
                    )
                # bias add (per-partition scalar) + cast to bf16, PSUM -> SBUF
                nc.vector.tensor_scalar(
                    out=dsts[ft // H][:, :, ft % H],
                    in0=ps,
                    scalar1=bias_sb[:, ft:ft + 1],
                    scalar2=None,
                    op0=mybir.AluOpType.add,
                )
                if ft == 8 and si + 1 < len(SLABS):
                    next_xt = fetch_xt(si + 1)
                # interleave previous slab's attention blocks across the ft loop
                while attn_j < nblk_prev * (ft + 1) // FT:
                    front = emit_attn_front(prev_tiles, attn_j)
                    if pending is not None:
                        emit_attn_back(pending)
                    pending = front
                    attn_j += 1
            if si > 0:
                nc.sync.dma_start(
                    zsum_out[:, prev_tiles[4] // 4:(prev_tiles[4] + SLABS[si - 1]) // 4],
                    prev_tiles[3],
                )
            prev_tiles = (q_sb, k_sb, v_sb, zs_sb, sstart)
            sstart += SL

        # last slab's attention (the only non-overlapped part)
        for blk in range(SLABS[-1] // 16):
            front = emit_attn_front(prev_tiles, blk)
            if pending is not None:
                emit_attn_back(pending)
            pending = front
        emit_attn_back(pending)
        nc.sync.dma_start(
            zsum_out[:, prev_tiles[4] // 4:(prev_tiles[4] + SLABS[-1]) // 4],
            prev_tiles[3],
        )

    nc.compile()
    return nc


def get_nc():
    global _CACHED_NC
    if _CACHED_NC is None:
        _CACHED_NC = build_nc()
    return _CACHED_NC


def prep_inputs(hidden_states, wq, bq, wk, bk, wv, bv):
    """Host-side layout prep. Returns per-core input maps."""
    bf16 = ml_dtypes.bfloat16

    # X^T tiled: [ipart, kt, p] with p the global position index
    xt_all = (
        np.ascontiguousarray(hidden_states.reshape(P_TOT, E).T)
        .astype(bf16)
        .reshape(KT, 128, P_TOT)
        .transpose(1, 0, 2)
    )  # [128, KT, 8192] (view)

    # Fused weight W[12288, 4096] -> W^T tiled [ft, ipart, kt*128 + f].
    # Weights are fp8-e4m3 roundtripped, so fp8 storage is lossless.
    wcat = np.concatenate([wq, wk, wv], axis=0)  # [3E, E]
    wt = (
        np.ascontiguousarray(wcat.T)
        .astype(ml_dtypes.float8_e4m3)
        .reshape(KT, 128, FT, 128)
        .transpose(2, 1, 0, 3)
    )
    wt = np.ascontiguousarray(wt).reshape(FT, 128, KT * 128)

    bias_cols = np.ascontiguousarray(
        np.concatenate([bq, bk, bv]).astype(np.float32).reshape(FT, 128).T
    )  # [128, FT]

    mask = np.zeros((128, 128), dtype=bf16)
    for p in range(4):
        mask[32 * p:32 * p + 32, 32 * p:32 * p + 32] = 1.0

    in_maps = []
    for c in range(N_CORES):
        xt_c = np.ascontiguousarray(xt_all[:, :, c * P_CORE:(c + 1) * P_CORE])
        in_maps.append({"xt": xt_c, "wt": wt, "bias": bias_cols, "maskd": mask})
    return in_maps


def normalize_shard(ctx_u, zsum):
    """ctx_u [128, P_CORE, H] bf16 (d, pos, h) unnormalized; zsum [128, P_CORE//4].

    Returns normalized [P_CORE, E] fp32. z for position pos, head h lives at
    zsum[32*(pos%4)+h, pos//4]."""
    ctx = np.asarray(ctx_u).astype(np.float32).transpose(1, 2, 0)  # [pos, h, d]
    z = np.asarray(zsum).astype(np.float32)  # [128, P_CORE//4]
    z = z.reshape(4, 32, P_CORE // 4).transpose(2, 0, 1).reshape(P_CORE, 32)
    return (ctx / z[:, :, None]).reshape(P_CORE, E)


def assemble_output(ctxs, zsums):
    out = np.empty((P_TOT, E), dtype=np.float32)
    for c in range(N_CORES):
        out[c * P_CORE:(c + 1) * P_CORE] = normalize_shard(ctxs[c], zsums[c])
    return out.reshape(B, S, E)


def kernel(**inputs):
    from concourse.bass_utils import run_bass_kernel_spmd

    nc = get_nc()
    in_maps = prep_inputs(
        inputs["hidden_states"],
        inputs["wq"], inputs["bq"],
        inputs["wk"], inputs["bk"],
        inputs["wv"], inputs["bv"],
    )
    res = run_bass_kernel_spmd(nc, in_maps, core_ids=list(range(N_CORES)))
    ctxs = [np.asarray(r["ctx"]).reshape(128, P_CORE, H) for r in res.results]
    zsums = [np.asarray(r["zsum"]).reshape(128, P_CORE // 4) for r in res.results]
    return assemble_output(ctxs, zsums)
